# revision 64
# baseline (speedup 1.0000x reference)
"""Linformer attention TRN2 kernel v6 (8-core SPMD, batch x head-group sharded).

Difference vs v5: q is projected explicitly as qT = Wq^T x^T DURING the
DMA-bound input window (it needs only Wq and the xT stream, not the E/F
reduction), so pass B's dots shrink to one 128-contraction matmul per
(superblock, head-pair) against the block-diagonal klrT.  Same FLOPs, but
~55us of matmul work moves off the serial pass-B critical path into the
window where the PE used to idle waiting on DMA.

Phase structure per core (b, heads hs..hs+8):
  interleaved streams:  xEF += EF_chunk^T @ x_chunk   (pass A, fp32 psum)
                        qT_t = sum_j Wq_j^T @ xT_j  (+bq via DVE)  [qc, n]
  A2: klrT/vlrT direct -> kbd/vbd (pair block-diag, +rank-1 bias), vw
  pass B per 512-row superblock:
    dotsT_t = kbd_t^T @ qT_t          (1 matmul, contract 128)
    expT_t  = Exp(0.125*dotsT - 80)   (ACT, bf16)
    sums_t  = blockones^T @ expT_t    (per-head partition sums, broadcast)
    attnT_t = expT_t * recip(sums_t)  (DVE, fp16)
    out_q  += attnT_tiles^T @ vw      (fp16, fp32 accum)
"""

import sys

import numpy as np

try:
    import concourse.bass as bass  # noqa: F401
except ImportError:
    sys.path.insert(0, "/opt/trn_rl_repo")

from contextlib import ExitStack

import concourse.bass as bass
import concourse.tile as tile
from concourse import bacc, mybir
from concourse.bass_utils import run_bass_kernel_spmd

N, B, DIM, H, K, DH = 4096, 4, 1024, 16, 64, 64
NH = 8
QC = NH * DH     # 512
NCORES = 8
NCHUNK = N // 128
NSUPER = 8
FP32 = mybir.dt.float32
FP16 = mybir.dt.float16
BF16 = mybir.dt.bfloat16

_PROG_CACHE = {}


def build_program():
    if "nc" in _PROG_CACHE:
        return _PROG_CACHE["nc"]
    nc = bacc.Bacc("TRN2", target_bir_lowering=False, debug=False)

    ident = nc.dram_tensor("ident", [128, 128], FP16, kind="ExternalInput")
    bones = nc.dram_tensor("bones", [128, 128], BF16, kind="ExternalInput")
    EFp = nc.dram_tensor("EFp", [128, NCHUNK * 2 * K], FP16, kind="ExternalInput")
    x_nat = nc.dram_tensor("x_nat", [N, DIM], FP16, kind="ExternalInput")
    xT = nc.dram_tensor("xT", [DIM, N], FP16, kind="ExternalInput")
    Wq = nc.dram_tensor("Wq", [DIM, QC], FP16, kind="ExternalInput")
    Wk = nc.dram_tensor("Wk", [DIM, QC], FP16, kind="ExternalInput")
    Wv = nc.dram_tensor("Wv", [DIM, QC], FP16, kind="ExternalInput")
    WoB = nc.dram_tensor("WoB", [QC, DIM], FP16, kind="ExternalInput")
    bqp = nc.dram_tensor("bqp", [128, 4], FP32, kind="ExternalInput")
    r1k = nc.dram_tensor("r1k", [128, 4 * K], FP32, kind="ExternalInput")
    r1v = nc.dram_tensor("r1v", [128, 4 * K], FP32, kind="ExternalInput")
    out_p = nc.dram_tensor("out_p", [N, DIM], FP16, kind="ExternalOutput")

    with tile.TileContext(nc) as tc, ExitStack() as ctx:
        singles = ctx.enter_context(tc.tile_pool(name="singles", bufs=1))

        # --- prologue: scalar queue carries Wq + small tensors early ---
        wq_t = singles.tile([128, 8, QC], FP16)
        nc.scalar.dma_start(wq_t[:], Wq[:].rearrange("(j p) c -> p j c", p=128))
        ident_t = singles.tile([128, 128], FP16)
        nc.scalar.dma_start(ident_t[:], ident[:])
        bones_t = singles.tile([128, 128], BF16)
        nc.scalar.dma_start(bones_t[:], bones[:])
        bqp_t = singles.tile([128, 4], FP32)
        nc.scalar.dma_start(bqp_t[:], bqp[:])
        r1k_t = singles.tile([128, 4, K], FP32)
        nc.scalar.dma_start(r1k_t[:], r1k[:].rearrange("p (t k) -> p t k", t=4))
        r1v_t = singles.tile([128, 4, K], FP32)
        nc.scalar.dma_start(r1v_t[:], r1v[:].rearrange("p (t k) -> p t k", t=4))

        ef_t = singles.tile([128, NCHUNK, 2 * K], FP16)

        wm_src = singles.tile([128, 512], FP16)
        nc.vector.memset(wm_src[:], 1.0)
        nbias = singles.tile([128, 1], FP32)
        nc.vector.memset(nbias[:], -80.0)
        warm_cm = tc.tile_pool(name="warm", bufs=1, space="PSUM")
        warm_pool = warm_cm.__enter__()
        wm_ps = warm_pool.tile([128, 512], FP32)

        def filler_mms(n):
            for _ in range(n):
                nc.tensor.matmul(wm_ps[:], wm_src[:, 0:128], wm_src[:],
                                 start=True, stop=True)

        filler_mms(23)

        # --- phase 1: interleaved xEF reduction + qT projection + A2 ---
        # x chunks are front-loaded (6 per group) so the E/F reduction ends
        # ~60% through the phase; A2's cross-engine chain then interleaves
        # with the remaining qT blocks, which hide its DVE/ACT latency.
        qT_sb = singles.tile([128, 4, N], FP16)   # [qc-pair-local, t, n]
        a2sb = ctx.enter_context(tc.tile_pool(name="a2sb", bufs=1))
        xe16 = a2sb.tile([64, DIM], FP16)
        xf16 = a2sb.tile([64, DIM], FP16)
        wk_t = singles.tile([128, 8, QC], FP16)
        wv_t = singles.tile([128, 8, QC], FP16)
        wob_t = singles.tile([128, 4, DIM], FP16)
        kbd = a2sb.tile([128, 4, 128], FP16)
        nc.vector.memset(kbd[:], 0.0)
        vbd = a2sb.tile([128, 4, 128], FP16)
        nc.vector.memset(vbd[:], 0.0)
        vw_sb = a2sb.tile([128, 4, DIM], FP16)

        a2_cm = tc.tile_pool(name="a2ps", bufs=1, space="PSUM")
        a2ps = a2_cm.__enter__()

        with tc.tile_pool(name="xef_ps", bufs=1, space="PSUM") as xef_pool, \
             tc.tile_pool(name="qt_ps", bufs=2, space="PSUM") as qt_pool, \
             tc.tile_pool(name="xa", bufs=8) as xa_pool, \
             tc.tile_pool(name="xts", bufs=3) as xts_pool:
            xef_ps = xef_pool.tile([128, DIM], FP32)
            nchunks_done = 0

            def emit_chunks(n):
                nonlocal nchunks_done
                for i in range(nchunks_done, min(nchunks_done + n, NCHUNK)):
                    x_t = xa_pool.tile([128, DIM], FP16)
                    nc.sync.dma_start(x_t[:], x_nat[i * 128:(i + 1) * 128, :])
                    for f in (0, 512):
                        nc.tensor.matmul(xef_ps[:, f:f + 512], ef_t[:, i, :],
                                         x_t[:, f:f + 512],
                                         start=(i == 0), stop=(i == NCHUNK - 1))
                nchunks_done = min(nchunks_done + n, NCHUNK)

            xts_tiles = {}

            def qt_dma(s):
                xts = xts_pool.tile([128, 8, 512], FP16)
                if s == 0:
                    # two slices: qT(0) starts on the first half while the
                    # second loads, and only ~1us of trigger time sits ahead
                    # of the x chunks on the queue
                    for j in (0, 4):
                        nc.sync.dma_start(
                            xts[:, j:j + 4, :],
                            xT[j * 128:(j + 4) * 128, 0:512]
                            .rearrange("(j p) n -> p j n", p=128))
                else:
                    nc.sync.dma_start(
                        xts[:],
                        xT[:, s * 512:(s + 1) * 512].rearrange("(j p) n -> p j n", p=128),
                    )
                xts_tiles[s] = xts
                if s == 6:
                    nc.sync.dma_start(
                        wk_t[:], Wk[:].rearrange("(j p) c -> p j c", p=128))
                    nc.sync.dma_start(
                        wv_t[:], Wv[:].rearrange("(j p) c -> p j c", p=128))
                if s == 7:
                    nc.sync.dma_start(
                        wob_t[:], WoB[:].rearrange("(t p) c -> p t c", p=128))

            def qt_ts(s, ts_list):
                if s not in xts_tiles:
                    qt_dma(s)
                xts = xts_tiles[s]
                for t in ts_list:
                    qt_ps = qt_pool.tile([128, 512], FP32)
                    for j in range(8):
                        nc.tensor.matmul(
                            qt_ps[:], wq_t[:, j, t * 128:(t + 1) * 128],
                            xts[:, j, :],
                            start=(j == 0), stop=(j == 7),
                        )
                    nc.vector.tensor_scalar_add(
                        out=qT_sb[:, t, s * 512:(s + 1) * 512],
                        in0=qt_ps[:], scalar1=bqp_t[:, t:t + 1],
                    )

            def qt_block(s):
                qt_ts(s, range(4))

            # xts0 ahead of everything so qT(0) fills the wait for x chunk 0
            qt_dma(0)
            nc.sync.dma_start(
                ef_t[:], EFp[:].rearrange("p (i k) -> p i k", i=NCHUNK))
            qt_block(0)
            for s in range(1, 6):
                emit_chunks(6)
                qt_block(s)
            emit_chunks(2)       # chunks 30, 31 -> xEF complete
            # evacuate xEF per-column-chunk
            for j in range(8):
                cs = slice(j * 128, (j + 1) * 128)
                nc.vector.tensor_copy(xe16[:, cs], xef_ps[0:64, cs])
                nc.scalar.copy(xf16[:, cs], xef_ps[64:128, cs])

            # A2 step 1: transpose xE/xF (qT blocks hide cross-engine latency)
            xet_sb = a2sb.tile([128, 8, K], FP16)
            xft_sb = a2sb.tile([128, 8, K], FP16)
            klr16 = a2sb.tile([64, QC], FP16)
            vlr16 = a2sb.tile([64, QC], FP16)

            def side_a(src, dst, on_act=False):
                tp = a2ps.tile([128, 8, K], FP16, tag="xt_ps")
                for j in range(8):
                    nc.tensor.transpose(
                        tp[:, j, :], src[:, j * 128:(j + 1) * 128],
                        ident_t[0:64, 0:64],
                    )
                if on_act:
                    nc.scalar.copy(dst[:], tp[:])
                else:
                    nc.vector.tensor_copy(dst[:], tp[:])

            def side_b_mm(w_t, xt, lr16, on_act=False):
                lpb = a2ps.tile([128, QC], FP32, tag="big")
                lp = lpb[0:64, :]
                for j in range(8):
                    nc.tensor.matmul(lp[:], xt[:, j, :], w_t[:, j, :],
                                     start=(j == 0), stop=(j == 7))
                if on_act:
                    nc.scalar.copy(lr16[:], lp[:])
                else:
                    nc.vector.tensor_copy(lr16[:], lp[:])

            def side_b_fin(r1, dst, lr16):
                tpl = a2ps.tile([128, 4, K], FP16, tag="lrt")
                for t in range(4):
                    nc.tensor.transpose(
                        tpl[:, t, :], lr16[:, t * 128:(t + 1) * 128],
                        ident_t[0:64, 0:64],
                    )
                for t in range(4):
                    nc.vector.tensor_add(out=dst[0:64, t, 0:64],
                                         in0=tpl[0:64, t, :], in1=r1[0:64, t, :])
                    nc.vector.tensor_add(out=dst[64:128, t, 64:128],
                                         in0=tpl[64:128, t, :], in1=r1[64:128, t, :])

            # weave the A2 chain with qT(6)/qT(7) pieces: every cross-engine
            # hop is covered by ~1.7us of independent PE work
            qt_ts(6, [0])
            side_a(xe16, xet_sb)
            qt_ts(6, [1])
            side_a(xf16, xft_sb, on_act=True)
            side_b_mm(wk_t, xet_sb, klr16)
            qt_ts(6, [2])
            side_b_fin(r1k_t, kbd, klr16)
            qt_ts(6, [3])
            side_b_mm(wv_t, xft_sb, vlr16, on_act=True)
            qt_ts(7, [0, 1])
            side_b_fin(r1v_t, vbd, vlr16)
            qt_ts(7, [2, 3])

        a2_cm.__exit__(None, None, None)
        warm_cm.__exit__(None, None, None)

        # A2 step 3: vw (own psum pool; pass B's back_half needs it ~17us out)
        with tc.tile_pool(name="vwps", bufs=2, space="PSUM") as vw_pool:
            for t in range(4):
                for f in (0, 512):
                    vw_ps = vw_pool.tile([128, 512], FP32)
                    nc.tensor.matmul(vw_ps[:], vbd[:, t, :],
                                     wob_t[:, t, f:f + 512],
                                     start=True, stop=True)
                    if f == 0:
                        nc.vector.tensor_copy(vw_sb[:, t, f:f + 512], vw_ps[:])
                    else:
                        nc.scalar.copy(vw_sb[:, t, f:f + 512], vw_ps[:])

        # ---------------- Pass B: dotsT -> softmax -> out --------------------
        # three-deep software pipeline: dots(s) | sums(s-1) | out(s-2), so the
        # PE never waits on the ACT exp or the DVE normalize
        exp_pool = ctx.enter_context(tc.tile_pool(name="expp", bufs=9))
        attn_pool = ctx.enter_context(tc.tile_pool(name="attn", bufs=3))
        osb_pool = ctx.enter_context(tc.tile_pool(name="osb", bufs=3))
        dots_pool = ctx.enter_context(tc.tile_pool(name="dots", bufs=4, space="PSUM"))
        sums_pool = ctx.enter_context(tc.tile_pool(name="sums", bufs=1, space="PSUM"))
        out_ps_pool = ctx.enter_context(tc.tile_pool(name="outps", bufs=2, space="PSUM"))

        exp_tiles = [None] * NSUPER
        attn_tiles = [None] * NSUPER

        def front_dots(s):
            exps = []
            for t in range(4):
                dots_ps = dots_pool.tile([128, 512], FP32)
                nc.tensor.matmul(dots_ps[:], kbd[:, t, :],
                                 qT_sb[:, t, s * 512:(s + 1) * 512],
                                 start=True, stop=True)
                expT = exp_pool.tile([128, 512], BF16)
                nc.scalar.activation(
                    out=expT[:], in_=dots_ps[:],
                    func=mybir.ActivationFunctionType.Exp,
                    scale=0.125, bias=nbias[:],
                )
                exps.append(expT)
            exp_tiles[s] = exps

        def front_sums(s):
            attnT = attn_pool.tile([128, 4, 512], FP16)
            attn_tiles[s] = attnT
            for t in range(4):
                expT = exp_tiles[s][t]
                sums_ps = sums_pool.tile([128, 512], FP32)
                nc.tensor.matmul(sums_ps[:], bones_t[:], expT[:],
                                 start=True, stop=True)
                rec = exp_pool.tile([128, 512], FP32, tag="rec")
                nc.vector.reciprocal_approx_fast(out=rec[:], in_=sums_ps[:])
                nc.vector.tensor_mul(out=attnT[:, t, :], in0=expT[:], in1=rec[:])

        def back_half(s):
            attnT = attn_tiles[s]
            for q in range(4):
                out_sb = osb_pool.tile([128, DIM], FP16)
                ps0 = out_ps_pool.tile([128, 512], FP32, tag="outps")
                ps1 = out_ps_pool.tile([128, 512], FP32, tag="outps")
                # t-outer: each attnT stationary is loaded once and serves
                # both 512-column halves (halves the LDWEIGHTS count)
                for t in range(4):
                    for f, out_ps in ((0, ps0), (512, ps1)):
                        nc.tensor.matmul(
                            out_ps[:],
                            attnT[:, t, q * 128:(q + 1) * 128],
                            vw_sb[:, t, f:f + 512],
                            start=(t == 0), stop=(t == 3),
                        )
                for f, out_ps in ((0, ps0), (512, ps1)):
                    # DVE is the trailing engine in pass B (recip+mult), so
                    # out evacuation lives on ACT; the final superblock keeps
                    # the DVE/ACT split to shorten the last copy->DMA chain
                    if s == NSUPER - 1 and f == 0:
                        nc.vector.tensor_copy(out_sb[:, 0:512], out_ps[:])
                    else:
                        nc.scalar.copy(out_sb[:, f:f + 512], out_ps[:])
                i = s * 4 + q
                # sync queue is idle once inputs are loaded; its hardware DGE
                # is much faster than gpsimd's software path
                nc.sync.dma_start(out_p[i * 128:(i + 1) * 128, :], out_sb[:])

        front_dots(0)
        front_dots(1)
        front_sums(0)
        for s in range(2, NSUPER):
            front_dots(s)
            front_sums(s - 1)
            back_half(s - 2)
        front_sums(NSUPER - 1)
        back_half(NSUPER - 2)
        back_half(NSUPER - 1)

    nc.finalize()
    _PROG_CACHE["nc"] = nc
    return nc


def shard_inputs(x, E, F, W_qkv, b_qkv, W_out, b_out):
    x = np.asarray(x, dtype=np.float32)
    E = np.asarray(E, dtype=np.float32)
    F = np.asarray(F, dtype=np.float32)
    W_qkv = np.asarray(W_qkv, dtype=np.float32)
    b_qkv = np.asarray(b_qkv, dtype=np.float32)
    W_out = np.asarray(W_out, dtype=np.float32)

    sE = E.sum(0).astype(np.float32)
    sF = F.sum(0).astype(np.float32)
    EF = np.concatenate([E, F], axis=1).astype(np.float16)
    EFp = np.ascontiguousarray(
        EF.reshape(NCHUNK, 128, 2 * K).transpose(1, 0, 2).reshape(128, -1))

    ident = np.eye(128, dtype=np.float16)
    import ml_dtypes
    bones = np.zeros((128, 128), np.float32)
    bones[:64, :64] = 1.0
    bones[64:, 64:] = 1.0
    bones = bones.astype(ml_dtypes.bfloat16)

    in_maps = []
    xb_cache = {}
    for c in range(NCORES):
        b, hg = c // 2, c % 2
        hs = NH * hg
        if b not in xb_cache:
            xb16 = np.ascontiguousarray(x[:, b, :]).astype(np.float16)
            xT16 = np.ascontiguousarray(xb16.T)
            xb_cache[b] = (xb16, xT16)
        xb16, xT16 = xb_cache[b]

        qcols = slice(hs * DH, (hs + NH) * DH)
        kcols = slice(DIM + hs * DH, DIM + (hs + NH) * DH)
        vcols = slice(2 * DIM + hs * DH, 2 * DIM + (hs + NH) * DH)

        bqp32 = np.ascontiguousarray(
            b_qkv[qcols].reshape(4, 128).T).astype(np.float32)
        bk = b_qkv[kcols]
        bv = b_qkv[vcols]
        r1kT = np.ascontiguousarray(
            (bk.reshape(4, 128)[:, :, None] * sE[None, None, :])
            .transpose(1, 0, 2).reshape(128, 4 * K))
        r1vT = np.ascontiguousarray(
            (bv.reshape(4, 128)[:, :, None] * sF[None, None, :])
            .transpose(1, 0, 2).reshape(128, 4 * K))

        in_maps.append({
            "ident": ident,
            "bones": bones,
            "EFp": EFp,
            "x_nat": xb16,
            "xT": xT16,
            "Wq": W_qkv[:, qcols].astype(np.float16),
            "Wk": W_qkv[:, kcols].astype(np.float16),
            "Wv": W_qkv[:, vcols].astype(np.float16),
            "WoB": W_out[hs * DH:(hs + NH) * DH, :].astype(np.float16),
            "bqp": bqp32,
            "r1k": r1kT.astype(np.float32),
            "r1v": r1vT.astype(np.float32),
        })
    return in_maps


def kernel_impl(inputs, trace=False, **run_kwargs):
    nc = build_program()
    in_maps = shard_inputs(
        inputs["x"], inputs["E"], inputs["F"], inputs["W_qkv"],
        inputs["b_qkv"], inputs["W_out"], inputs["b_out"],
    )
    res = run_bass_kernel_spmd(nc, in_maps, list(range(NCORES)),
                               trace=trace, **run_kwargs)
    b_out = np.asarray(inputs["b_out"], dtype=np.float32)
    out = np.empty((N, B, DIM), np.float32)
    for b in range(B):
        out[:, b, :] = (res.results[2 * b]["out_p"].astype(np.float32)
                        + res.results[2 * b + 1]["out_p"].astype(np.float32)
                        + b_out)
    return out, res


def kernel(**inputs):
    out, _ = kernel_impl(inputs)
    return out


# revision 66
# speedup vs baseline: 1.0580x; 1.0580x over previous
"""Linformer attention TRN2 kernel v6 (8-core SPMD, batch x head-group sharded).

Difference vs v5: q is projected explicitly as qT = Wq^T x^T DURING the
DMA-bound input window (it needs only Wq and the xT stream, not the E/F
reduction), so pass B's dots shrink to one 128-contraction matmul per
(superblock, head-pair) against the block-diagonal klrT.  Same FLOPs, but
~55us of matmul work moves off the serial pass-B critical path into the
window where the PE used to idle waiting on DMA.

Phase structure per core (b, heads hs..hs+8):
  interleaved streams:  xEF += EF_chunk^T @ x_chunk   (pass A, fp32 psum)
                        qT_t = sum_j Wq_j^T @ xT_j  (+bq via DVE)  [qc, n]
  A2: klrT/vlrT direct -> kbd/vbd (pair block-diag, +rank-1 bias), vw
  pass B per 512-row superblock:
    dotsT_t = kbd_t^T @ qT_t          (1 matmul, contract 128)
    expT_t  = Exp(0.125*dotsT - 80)   (ACT, bf16)
    sums_t  = blockones^T @ expT_t    (per-head partition sums, broadcast)
    attnT_t = expT_t * recip(sums_t)  (DVE, fp16)
    out_q  += attnT_tiles^T @ vw      (fp16, fp32 accum)
"""

import sys

import numpy as np

try:
    import concourse.bass as bass  # noqa: F401
except ImportError:
    sys.path.insert(0, "/opt/trn_rl_repo")

from contextlib import ExitStack

import concourse.bass as bass
import concourse.tile as tile
from concourse import bacc, mybir
from concourse.bass_utils import run_bass_kernel_spmd

N, B, DIM, H, K, DH = 4096, 4, 1024, 16, 64, 64
NH = 8
QC = NH * DH     # 512
NCORES = 8
NCHUNK = N // 128
NSUPER = 8
FP32 = mybir.dt.float32
FP16 = mybir.dt.float16
BF16 = mybir.dt.bfloat16

_PROG_CACHE = {}


def build_program():
    if "nc" in _PROG_CACHE:
        return _PROG_CACHE["nc"]
    nc = bacc.Bacc("TRN2", target_bir_lowering=False, debug=False)

    ident = nc.dram_tensor("ident", [128, 128], FP16, kind="ExternalInput")
    bones = nc.dram_tensor("bones", [128, 128], BF16, kind="ExternalInput")
    EFp = nc.dram_tensor("EFp", [128, NCHUNK * 2 * K], FP16, kind="ExternalInput")
    x_nat = nc.dram_tensor("x_nat", [N, DIM], FP16, kind="ExternalInput")
    xT = nc.dram_tensor("xT", [DIM, N], FP16, kind="ExternalInput")
    Wq = nc.dram_tensor("Wq", [DIM, QC], FP16, kind="ExternalInput")
    Wk = nc.dram_tensor("Wk", [DIM, QC], FP16, kind="ExternalInput")
    Wv = nc.dram_tensor("Wv", [DIM, QC], FP16, kind="ExternalInput")
    WoB = nc.dram_tensor("WoB", [QC, DIM], FP16, kind="ExternalInput")
    bqp = nc.dram_tensor("bqp", [128, 4], FP32, kind="ExternalInput")
    r1k = nc.dram_tensor("r1k", [128, 4 * K], FP32, kind="ExternalInput")
    r1v = nc.dram_tensor("r1v", [128, 4 * K], FP32, kind="ExternalInput")
    out_p = nc.dram_tensor("out_p", [N, DIM], FP16, kind="ExternalOutput")

    with tile.TileContext(nc) as tc, ExitStack() as ctx:
        singles = ctx.enter_context(tc.tile_pool(name="singles", bufs=1))

        # --- prologue: scalar queue carries Wq + small tensors early ---
        wq_t = singles.tile([128, 8, QC], FP16)
        nc.scalar.dma_start(wq_t[:], Wq[:].rearrange("(j p) c -> p j c", p=128))
        ident_t = singles.tile([128, 128], FP16)
        nc.scalar.dma_start(ident_t[:], ident[:])
        bones_t = singles.tile([128, 128], BF16)
        nc.scalar.dma_start(bones_t[:], bones[:])
        bqp_t = singles.tile([128, 4], FP32)
        nc.scalar.dma_start(bqp_t[:], bqp[:])
        r1k_t = singles.tile([128, 4, K], FP32)
        nc.scalar.dma_start(r1k_t[:], r1k[:].rearrange("p (t k) -> p t k", t=4))
        r1v_t = singles.tile([128, 4, K], FP32)
        nc.scalar.dma_start(r1v_t[:], r1v[:].rearrange("p (t k) -> p t k", t=4))

        ef_t = singles.tile([128, NCHUNK, 2 * K], FP16)

        wm_src = singles.tile([128, 512], FP16)
        nc.vector.memset(wm_src[:], 1.0)
        nbias = singles.tile([128, 1], FP32)
        nc.vector.memset(nbias[:], -80.0)
        warm_cm = tc.tile_pool(name="warm", bufs=1, space="PSUM")
        warm_pool = warm_cm.__enter__()
        wm_ps = warm_pool.tile([128, 512], FP32)

        def filler_mms(n):
            for _ in range(n):
                nc.tensor.matmul(wm_ps[:], wm_src[:, 0:128], wm_src[:],
                                 start=True, stop=True)

        filler_mms(23)

        # --- phase 1: interleaved xEF reduction + qT projection + A2 ---
        # x chunks are front-loaded (6 per group) so the E/F reduction ends
        # ~60% through the phase; A2's cross-engine chain then interleaves
        # with the remaining qT blocks, which hide its DVE/ACT latency.
        qT_sb = singles.tile([128, 4, N], FP16)   # [qc-pair-local, t, n]
        a2sb = ctx.enter_context(tc.tile_pool(name="a2sb", bufs=1))
        xe16 = a2sb.tile([64, DIM], FP16)
        xf16 = a2sb.tile([64, DIM], FP16)
        wk_t = singles.tile([128, 8, QC], FP16)
        wv_t = singles.tile([128, 8, QC], FP16)
        wob_t = singles.tile([128, 4, DIM], FP16)
        kbd = a2sb.tile([128, 4, 128], FP16)
        nc.vector.memset(kbd[:], 0.0)
        vbd = a2sb.tile([128, 4, 128], FP16)
        nc.vector.memset(vbd[:], 0.0)
        vw_sb = a2sb.tile([128, 4, DIM], FP16)

        a2_cm = tc.tile_pool(name="a2ps", bufs=1, space="PSUM")
        a2ps = a2_cm.__enter__()

        with tc.tile_pool(name="xef_ps", bufs=1, space="PSUM") as xef_pool, \
             tc.tile_pool(name="qt_ps", bufs=2, space="PSUM") as qt_pool, \
             tc.tile_pool(name="xa", bufs=8) as xa_pool, \
             tc.tile_pool(name="xts", bufs=3) as xts_pool:
            xef_ps = xef_pool.tile([128, DIM], FP32)
            nchunks_done = 0

            def emit_chunks(n):
                nonlocal nchunks_done
                for i in range(nchunks_done, min(nchunks_done + n, NCHUNK)):
                    x_t = xa_pool.tile([128, DIM], FP16)
                    nc.sync.dma_start(x_t[:], x_nat[i * 128:(i + 1) * 128, :])
                    for f in (0, 512):
                        nc.tensor.matmul(xef_ps[:, f:f + 512], ef_t[:, i, :],
                                         x_t[:, f:f + 512],
                                         start=(i == 0), stop=(i == NCHUNK - 1))
                nchunks_done = min(nchunks_done + n, NCHUNK)

            xts_tiles = {}

            def qt_dma(s):
                xts = xts_pool.tile([128, 8, 512], FP16)
                if s == 0:
                    # two slices: qT(0) starts on the first half while the
                    # second loads, and only ~1us of trigger time sits ahead
                    # of the x chunks on the queue
                    for j in (0, 4):
                        nc.sync.dma_start(
                            xts[:, j:j + 4, :],
                            xT[j * 128:(j + 4) * 128, 0:512]
                            .rearrange("(j p) n -> p j n", p=128))
                else:
                    nc.sync.dma_start(
                        xts[:],
                        xT[:, s * 512:(s + 1) * 512].rearrange("(j p) n -> p j n", p=128),
                    )
                xts_tiles[s] = xts
                if s == 6:
                    nc.sync.dma_start(
                        wk_t[:], Wk[:].rearrange("(j p) c -> p j c", p=128))
                    nc.sync.dma_start(
                        wv_t[:], Wv[:].rearrange("(j p) c -> p j c", p=128))
                if s == 7:
                    nc.sync.dma_start(
                        wob_t[:], WoB[:].rearrange("(t p) c -> p t c", p=128))

            def qt_ts(s, ts_list):
                if s not in xts_tiles:
                    qt_dma(s)
                xts = xts_tiles[s]
                for t in ts_list:
                    qt_ps = qt_pool.tile([128, 512], FP32)
                    for j in range(8):
                        nc.tensor.matmul(
                            qt_ps[:], wq_t[:, j, t * 128:(t + 1) * 128],
                            xts[:, j, :],
                            start=(j == 0), stop=(j == 7),
                        )
                    nc.vector.tensor_scalar_add(
                        out=qT_sb[:, t, s * 512:(s + 1) * 512],
                        in0=qt_ps[:], scalar1=bqp_t[:, t:t + 1],
                    )

            def qt_block(s):
                qt_ts(s, range(4))

            # xts0 ahead of everything so qT(0) fills the wait for x chunk 0
            qt_dma(0)
            nc.sync.dma_start(
                ef_t[:], EFp[:].rearrange("p (i k) -> p i k", i=NCHUNK))
            qt_block(0)
            for s in range(1, 6):
                emit_chunks(6)
                qt_block(s)
            emit_chunks(2)       # chunks 30, 31 -> xEF complete
            # evacuate xEF per-column-chunk
            for j in range(8):
                cs = slice(j * 128, (j + 1) * 128)
                nc.vector.tensor_copy(xe16[:, cs], xef_ps[0:64, cs])
                nc.scalar.copy(xf16[:, cs], xef_ps[64:128, cs])

            # A2 step 1: transpose xE/xF (qT blocks hide cross-engine latency)
            xet_sb = a2sb.tile([128, 8, K], FP16)
            xft_sb = a2sb.tile([128, 8, K], FP16)
            klr16 = a2sb.tile([64, QC], FP16)
            vlr16 = a2sb.tile([64, QC], FP16)

            def side_a(src, dst, on_act=False):
                tp = a2ps.tile([128, 8, K], FP16, tag="xt_ps")
                for j in range(8):
                    nc.tensor.transpose(
                        tp[:, j, :], src[:, j * 128:(j + 1) * 128],
                        ident_t[0:64, 0:64],
                    )
                if on_act:
                    nc.scalar.copy(dst[:], tp[:])
                else:
                    nc.vector.tensor_copy(dst[:], tp[:])

            def side_b_mm(w_t, xt, lr16, on_act=False):
                lpb = a2ps.tile([128, QC], FP32, tag="big")
                lp = lpb[0:64, :]
                for j in range(8):
                    nc.tensor.matmul(lp[:], xt[:, j, :], w_t[:, j, :],
                                     start=(j == 0), stop=(j == 7))
                if on_act:
                    nc.scalar.copy(lr16[:], lp[:])
                else:
                    nc.vector.tensor_copy(lr16[:], lp[:])

            def side_b_fin(r1, dst, lr16):
                tpl = a2ps.tile([128, 4, K], FP16, tag="lrt")
                for t in range(4):
                    nc.tensor.transpose(
                        tpl[:, t, :], lr16[:, t * 128:(t + 1) * 128],
                        ident_t[0:64, 0:64],
                    )
                for t in range(4):
                    nc.vector.tensor_add(out=dst[0:64, t, 0:64],
                                         in0=tpl[0:64, t, :], in1=r1[0:64, t, :])
                    nc.vector.tensor_add(out=dst[64:128, t, 64:128],
                                         in0=tpl[64:128, t, :], in1=r1[64:128, t, :])

            # weave the A2 chain with qT(6)/qT(7) pieces: every cross-engine
            # hop is covered by ~1.7us of independent PE work
            qt_ts(6, [0])
            side_a(xe16, xet_sb)
            qt_ts(6, [1])
            side_a(xf16, xft_sb, on_act=True)
            side_b_mm(wk_t, xet_sb, klr16)
            qt_ts(6, [2])
            side_b_fin(r1k_t, kbd, klr16)
            qt_ts(6, [3])
            side_b_mm(wv_t, xft_sb, vlr16, on_act=True)
            qt_ts(7, [0, 1])
            side_b_fin(r1v_t, vbd, vlr16)
            qt_ts(7, [2, 3])

        a2_cm.__exit__(None, None, None)
        warm_cm.__exit__(None, None, None)

        # A2 step 3: vw (own psum pool; pass B's back_half needs it ~17us out)
        with tc.tile_pool(name="vwps", bufs=2, space="PSUM") as vw_pool:
            for t in range(4):
                for f in (0, 512):
                    vw_ps = vw_pool.tile([128, 512], FP32)
                    nc.tensor.matmul(vw_ps[:], vbd[:, t, :],
                                     wob_t[:, t, f:f + 512],
                                     start=True, stop=True)
                    if f == 0:
                        nc.vector.tensor_copy(vw_sb[:, t, f:f + 512], vw_ps[:])
                    else:
                        nc.scalar.copy(vw_sb[:, t, f:f + 512], vw_ps[:])

        # ---------------- Pass B: dotsT -> softmax -> out --------------------
        # three-deep software pipeline: dots(s) | sums(s-1) | out(s-2), so the
        # PE never waits on the ACT exp or the DVE normalize
        exp_pool = ctx.enter_context(tc.tile_pool(name="expp", bufs=9))
        attn_pool = ctx.enter_context(tc.tile_pool(name="attn", bufs=3))
        osb_pool = ctx.enter_context(tc.tile_pool(name="osb", bufs=3))
        dots_pool = ctx.enter_context(tc.tile_pool(name="dots", bufs=4, space="PSUM"))
        sums_pool = ctx.enter_context(tc.tile_pool(name="sums", bufs=1, space="PSUM"))
        out_ps_pool = ctx.enter_context(tc.tile_pool(name="outps", bufs=2, space="PSUM"))

        exp_tiles = [None] * NSUPER
        attn_tiles = [None] * NSUPER

        def front_dots(s):
            exps = []
            for t in range(4):
                dots_ps = dots_pool.tile([128, 512], FP32)
                nc.tensor.matmul(dots_ps[:], kbd[:, t, :],
                                 qT_sb[:, t, s * 512:(s + 1) * 512],
                                 start=True, stop=True)
                expT = exp_pool.tile([128, 512], BF16)
                nc.scalar.activation(
                    out=expT[:], in_=dots_ps[:],
                    func=mybir.ActivationFunctionType.Exp,
                    scale=0.125, bias=nbias[:],
                )
                exps.append(expT)
            exp_tiles[s] = exps

        def front_sums(s):
            attnT = attn_pool.tile([128, 4, 512], FP16)
            attn_tiles[s] = attnT
            for t in range(4):
                expT = exp_tiles[s][t]
                sums_ps = sums_pool.tile([128, 512], FP32)
                nc.tensor.matmul(sums_ps[:], bones_t[:], expT[:],
                                 start=True, stop=True)
                rec = exp_pool.tile([128, 512], FP32, tag="rec")
                nc.vector.reciprocal_approx_fast(out=rec[:], in_=sums_ps[:])
                nc.vector.tensor_mul(out=attnT[:, t, :], in0=expT[:], in1=rec[:])

        def back_half(s):
            attnT = attn_tiles[s]
            for q in range(4):
                out_sb = osb_pool.tile([128, DIM], FP16)
                for f in (0, 512):
                    out_ps = out_ps_pool.tile([128, 512], FP32)
                    for t in range(4):
                        nc.tensor.matmul(
                            out_ps[:],
                            attnT[:, t, q * 128:(q + 1) * 128],
                            vw_sb[:, t, f:f + 512],
                            start=(t == 0), stop=(t == 3),
                        )
                    # DVE is the trailing engine in pass B (recip+mult), so
                    # out evacuation lives on ACT; the final superblock keeps
                    # the DVE/ACT split to shorten the last copy->DMA chain
                    if s == NSUPER - 1 and f == 0:
                        nc.vector.tensor_copy(out_sb[:, 0:512], out_ps[:])
                    else:
                        nc.scalar.copy(out_sb[:, f:f + 512], out_ps[:])
                i = s * 4 + q
                # sync queue is idle once inputs are loaded; its hardware DGE
                # is much faster than gpsimd's software path
                nc.sync.dma_start(out_p[i * 128:(i + 1) * 128, :], out_sb[:])

        front_dots(0)
        front_dots(1)
        front_sums(0)
        for s in range(2, NSUPER):
            front_dots(s)
            front_sums(s - 1)
            back_half(s - 2)
        front_sums(NSUPER - 1)
        back_half(NSUPER - 2)
        back_half(NSUPER - 1)

    nc.finalize()
    _PROG_CACHE["nc"] = nc
    return nc


def shard_inputs(x, E, F, W_qkv, b_qkv, W_out, b_out):
    x = np.asarray(x, dtype=np.float32)
    E = np.asarray(E, dtype=np.float32)
    F = np.asarray(F, dtype=np.float32)
    W_qkv = np.asarray(W_qkv, dtype=np.float32)
    b_qkv = np.asarray(b_qkv, dtype=np.float32)
    W_out = np.asarray(W_out, dtype=np.float32)

    sE = E.sum(0).astype(np.float32)
    sF = F.sum(0).astype(np.float32)
    EF = np.concatenate([E, F], axis=1).astype(np.float16)
    EFp = np.ascontiguousarray(
        EF.reshape(NCHUNK, 128, 2 * K).transpose(1, 0, 2).reshape(128, -1))

    ident = np.eye(128, dtype=np.float16)
    import ml_dtypes
    bones = np.zeros((128, 128), np.float32)
    bones[:64, :64] = 1.0
    bones[64:, 64:] = 1.0
    bones = bones.astype(ml_dtypes.bfloat16)

    in_maps = []
    xb_cache = {}
    for c in range(NCORES):
        b, hg = c // 2, c % 2
        hs = NH * hg
        if b not in xb_cache:
            xb16 = np.ascontiguousarray(x[:, b, :]).astype(np.float16)
            xT16 = np.ascontiguousarray(xb16.T)
            xb_cache[b] = (xb16, xT16)
        xb16, xT16 = xb_cache[b]

        qcols = slice(hs * DH, (hs + NH) * DH)
        kcols = slice(DIM + hs * DH, DIM + (hs + NH) * DH)
        vcols = slice(2 * DIM + hs * DH, 2 * DIM + (hs + NH) * DH)

        bqp32 = np.ascontiguousarray(
            b_qkv[qcols].reshape(4, 128).T).astype(np.float32)
        bk = b_qkv[kcols]
        bv = b_qkv[vcols]
        r1kT = np.ascontiguousarray(
            (bk.reshape(4, 128)[:, :, None] * sE[None, None, :])
            .transpose(1, 0, 2).reshape(128, 4 * K))
        r1vT = np.ascontiguousarray(
            (bv.reshape(4, 128)[:, :, None] * sF[None, None, :])
            .transpose(1, 0, 2).reshape(128, 4 * K))

        in_maps.append({
            "ident": ident,
            "bones": bones,
            "EFp": EFp,
            "x_nat": xb16,
            "xT": xT16,
            "Wq": W_qkv[:, qcols].astype(np.float16),
            "Wk": W_qkv[:, kcols].astype(np.float16),
            "Wv": W_qkv[:, vcols].astype(np.float16),
            "WoB": W_out[hs * DH:(hs + NH) * DH, :].astype(np.float16),
            "bqp": bqp32,
            "r1k": r1kT.astype(np.float32),
            "r1v": r1vT.astype(np.float32),
        })
    return in_maps


def kernel_impl(inputs, trace=False, **run_kwargs):
    nc = build_program()
    in_maps = shard_inputs(
        inputs["x"], inputs["E"], inputs["F"], inputs["W_qkv"],
        inputs["b_qkv"], inputs["W_out"], inputs["b_out"],
    )
    res = run_bass_kernel_spmd(nc, in_maps, list(range(NCORES)),
                               trace=trace, **run_kwargs)
    b_out = np.asarray(inputs["b_out"], dtype=np.float32)
    out = np.empty((N, B, DIM), np.float32)
    for b in range(B):
        out[:, b, :] = (res.results[2 * b]["out_p"].astype(np.float32)
                        + res.results[2 * b + 1]["out_p"].astype(np.float32)
                        + b_out)
    return out, res


def kernel(**inputs):
    out, _ = kernel_impl(inputs)
    return out


# revision 67
# speedup vs baseline: 1.0777x; 1.0185x over previous
"""Linformer attention TRN2 kernel v6 (8-core SPMD, batch x head-group sharded).

Difference vs v5: q is projected explicitly as qT = Wq^T x^T DURING the
DMA-bound input window (it needs only Wq and the xT stream, not the E/F
reduction), so pass B's dots shrink to one 128-contraction matmul per
(superblock, head-pair) against the block-diagonal klrT.  Same FLOPs, but
~55us of matmul work moves off the serial pass-B critical path into the
window where the PE used to idle waiting on DMA.

Phase structure per core (b, heads hs..hs+8):
  interleaved streams:  xEF += EF_chunk^T @ x_chunk   (pass A, fp32 psum)
                        qT_t = sum_j Wq_j^T @ xT_j  (+bq via DVE)  [qc, n]
  A2: klrT/vlrT direct -> kbd/vbd (pair block-diag, +rank-1 bias), vw
  pass B per 512-row superblock:
    dotsT_t = kbd_t^T @ qT_t          (1 matmul, contract 128)
    expT_t  = Exp(0.125*dotsT - 80)   (ACT, bf16)
    sums_t  = blockones^T @ expT_t    (per-head partition sums, broadcast)
    attnT_t = expT_t * recip(sums_t)  (DVE, fp16)
    out_q  += attnT_tiles^T @ vw      (fp16, fp32 accum)
"""

import sys

import numpy as np

try:
    import concourse.bass as bass  # noqa: F401
except ImportError:
    sys.path.insert(0, "/opt/trn_rl_repo")

from contextlib import ExitStack

import concourse.bass as bass
import concourse.tile as tile
from concourse import bacc, mybir
from concourse.bass_utils import run_bass_kernel_spmd

N, B, DIM, H, K, DH = 4096, 4, 1024, 16, 64, 64
NH = 8
QC = NH * DH     # 512
NCORES = 8
NCHUNK = N // 128
NSUPER = 8
FP32 = mybir.dt.float32
FP16 = mybir.dt.float16
BF16 = mybir.dt.bfloat16

_PROG_CACHE = {}


def build_program():
    if "nc" in _PROG_CACHE:
        return _PROG_CACHE["nc"]
    nc = bacc.Bacc("TRN2", target_bir_lowering=False, debug=False)

    ident = nc.dram_tensor("ident", [128, 128], FP16, kind="ExternalInput")
    bones = nc.dram_tensor("bones", [128, 128], BF16, kind="ExternalInput")
    EFp = nc.dram_tensor("EFp", [128, NCHUNK * 2 * K], FP16, kind="ExternalInput")
    x_nat = nc.dram_tensor("x_nat", [N, DIM], FP16, kind="ExternalInput")
    xT = nc.dram_tensor("xT", [DIM, N], FP16, kind="ExternalInput")
    Wq = nc.dram_tensor("Wq", [DIM, QC], FP16, kind="ExternalInput")
    Wk = nc.dram_tensor("Wk", [DIM, QC], FP16, kind="ExternalInput")
    Wv = nc.dram_tensor("Wv", [DIM, QC], FP16, kind="ExternalInput")
    WoB = nc.dram_tensor("WoB", [QC, DIM], FP16, kind="ExternalInput")
    bqp = nc.dram_tensor("bqp", [128, 4], FP32, kind="ExternalInput")
    r1k = nc.dram_tensor("r1k", [128, 4 * K], FP32, kind="ExternalInput")
    r1v = nc.dram_tensor("r1v", [128, 4 * K], FP32, kind="ExternalInput")
    out_p = nc.dram_tensor("out_p", [N, DIM], FP16, kind="ExternalOutput")

    with tile.TileContext(nc) as tc, ExitStack() as ctx:
        singles = ctx.enter_context(tc.tile_pool(name="singles", bufs=1))

        # --- prologue: scalar queue carries Wq + small tensors early ---
        wq_t = singles.tile([128, 8, QC], FP16)
        nc.scalar.dma_start(wq_t[:], Wq[:].rearrange("(j p) c -> p j c", p=128))
        ident_t = singles.tile([128, 128], FP16)
        nc.scalar.dma_start(ident_t[:], ident[:])
        bones_t = singles.tile([128, 128], BF16)
        nc.scalar.dma_start(bones_t[:], bones[:])
        bqp_t = singles.tile([128, 4], FP32)
        nc.scalar.dma_start(bqp_t[:], bqp[:])
        r1k_t = singles.tile([128, 4, K], FP32)
        nc.scalar.dma_start(r1k_t[:], r1k[:].rearrange("p (t k) -> p t k", t=4))
        r1v_t = singles.tile([128, 4, K], FP32)
        nc.scalar.dma_start(r1v_t[:], r1v[:].rearrange("p (t k) -> p t k", t=4))

        ef_t = singles.tile([128, NCHUNK, 2 * K], FP16)

        wm_src = singles.tile([128, 512], FP16)
        nc.vector.memset(wm_src[:], 1.0)
        nbias = singles.tile([128, 1], FP32)
        nc.vector.memset(nbias[:], -80.0)
        warm_cm = tc.tile_pool(name="warm", bufs=1, space="PSUM")
        warm_pool = warm_cm.__enter__()
        wm_ps = warm_pool.tile([128, 512], FP32)

        def filler_mms(n):
            for _ in range(n):
                nc.tensor.matmul(wm_ps[:], wm_src[:, 0:128], wm_src[:],
                                 start=True, stop=True)

        filler_mms(15)

        # --- phase 1: interleaved xEF reduction + qT projection + A2 ---
        # x chunks are front-loaded (6 per group) so the E/F reduction ends
        # ~60% through the phase; A2's cross-engine chain then interleaves
        # with the remaining qT blocks, which hide its DVE/ACT latency.
        qT_sb = singles.tile([128, 4, N], FP16)   # [qc-pair-local, t, n]
        a2sb = ctx.enter_context(tc.tile_pool(name="a2sb", bufs=1))
        xe16 = a2sb.tile([64, DIM], FP16)
        xf16 = a2sb.tile([64, DIM], FP16)
        wk_t = singles.tile([128, 8, QC], FP16)
        wv_t = singles.tile([128, 8, QC], FP16)
        wob_t = singles.tile([128, 4, DIM], FP16)
        kbd = a2sb.tile([128, 4, 128], FP16)
        nc.vector.memset(kbd[:], 0.0)
        vbd = a2sb.tile([128, 4, 128], FP16)
        nc.vector.memset(vbd[:], 0.0)
        vw_sb = a2sb.tile([128, 4, DIM], FP16)

        a2_cm = tc.tile_pool(name="a2ps", bufs=1, space="PSUM")
        a2ps = a2_cm.__enter__()

        with tc.tile_pool(name="xef_ps", bufs=1, space="PSUM") as xef_pool, \
             tc.tile_pool(name="qt_ps", bufs=2, space="PSUM") as qt_pool, \
             tc.tile_pool(name="xa", bufs=8) as xa_pool, \
             tc.tile_pool(name="xts", bufs=3) as xts_pool:
            xef_ps = xef_pool.tile([128, DIM], FP32)
            nchunks_done = 0

            def emit_chunks(n):
                nonlocal nchunks_done
                for i in range(nchunks_done, min(nchunks_done + n, NCHUNK)):
                    x_t = xa_pool.tile([128, DIM], FP16)
                    nc.sync.dma_start(x_t[:], x_nat[i * 128:(i + 1) * 128, :])
                    for f in (0, 512):
                        nc.tensor.matmul(xef_ps[:, f:f + 512], ef_t[:, i, :],
                                         x_t[:, f:f + 512],
                                         start=(i == 0), stop=(i == NCHUNK - 1))
                nchunks_done = min(nchunks_done + n, NCHUNK)

            xts_tiles = {}

            def qt_dma(s):
                xts = xts_pool.tile([128, 8, 512], FP16)
                if s == 0:
                    # two slices: qT(0) starts on the first half while the
                    # second loads, and only ~1us of trigger time sits ahead
                    # of the x chunks on the queue
                    for j in (0, 4):
                        nc.sync.dma_start(
                            xts[:, j:j + 4, :],
                            xT[j * 128:(j + 4) * 128, 0:512]
                            .rearrange("(j p) n -> p j n", p=128))
                else:
                    nc.sync.dma_start(
                        xts[:],
                        xT[:, s * 512:(s + 1) * 512].rearrange("(j p) n -> p j n", p=128),
                    )
                xts_tiles[s] = xts
                if s == 6:
                    nc.sync.dma_start(
                        wk_t[:], Wk[:].rearrange("(j p) c -> p j c", p=128))
                    nc.sync.dma_start(
                        wv_t[:], Wv[:].rearrange("(j p) c -> p j c", p=128))
                if s == 7:
                    nc.sync.dma_start(
                        wob_t[:], WoB[:].rearrange("(t p) c -> p t c", p=128))

            def qt_ts(s, ts_list):
                if s not in xts_tiles:
                    qt_dma(s)
                xts = xts_tiles[s]
                for t in ts_list:
                    qt_ps = qt_pool.tile([128, 512], FP32)
                    for j in range(8):
                        nc.tensor.matmul(
                            qt_ps[:], wq_t[:, j, t * 128:(t + 1) * 128],
                            xts[:, j, :],
                            start=(j == 0), stop=(j == 7),
                        )
                    nc.vector.tensor_scalar_add(
                        out=qT_sb[:, t, s * 512:(s + 1) * 512],
                        in0=qt_ps[:], scalar1=bqp_t[:, t:t + 1],
                    )

            def qt_block(s):
                qt_ts(s, range(4))

            # xts0 ahead of everything so qT(0) fills the wait for x chunk 0
            qt_dma(0)
            nc.sync.dma_start(
                ef_t[:], EFp[:].rearrange("p (i k) -> p i k", i=NCHUNK))
            qt_block(0)
            for s in range(1, 6):
                emit_chunks(6)
                qt_block(s)
            emit_chunks(2)       # chunks 30, 31 -> xEF complete
            # evacuate xEF per-column-chunk
            for j in range(8):
                cs = slice(j * 128, (j + 1) * 128)
                nc.vector.tensor_copy(xe16[:, cs], xef_ps[0:64, cs])
                nc.scalar.copy(xf16[:, cs], xef_ps[64:128, cs])

            # A2 step 1: transpose xE/xF (qT blocks hide cross-engine latency)
            xet_sb = a2sb.tile([128, 8, K], FP16)
            xft_sb = a2sb.tile([128, 8, K], FP16)
            klr16 = a2sb.tile([64, QC], FP16)
            vlr16 = a2sb.tile([64, QC], FP16)

            def side_a(src, dst, on_act=False):
                tp = a2ps.tile([128, 8, K], FP16, tag="xt_ps")
                for j in range(8):
                    nc.tensor.transpose(
                        tp[:, j, :], src[:, j * 128:(j + 1) * 128],
                        ident_t[0:64, 0:64],
                    )
                if on_act:
                    nc.scalar.copy(dst[:], tp[:])
                else:
                    nc.vector.tensor_copy(dst[:], tp[:])

            def side_b_mm(w_t, xt, lr16, on_act=False):
                lpb = a2ps.tile([128, QC], FP32, tag="big")
                lp = lpb[0:64, :]
                for j in range(8):
                    nc.tensor.matmul(lp[:], xt[:, j, :], w_t[:, j, :],
                                     start=(j == 0), stop=(j == 7))
                if on_act:
                    nc.scalar.copy(lr16[:], lp[:])
                else:
                    nc.vector.tensor_copy(lr16[:], lp[:])

            def side_b_fin(r1, dst, lr16):
                tpl = a2ps.tile([128, 4, K], FP16, tag="lrt")
                for t in range(4):
                    nc.tensor.transpose(
                        tpl[:, t, :], lr16[:, t * 128:(t + 1) * 128],
                        ident_t[0:64, 0:64],
                    )
                for t in range(4):
                    nc.vector.tensor_add(out=dst[0:64, t, 0:64],
                                         in0=tpl[0:64, t, :], in1=r1[0:64, t, :])
                    nc.vector.tensor_add(out=dst[64:128, t, 64:128],
                                         in0=tpl[64:128, t, :], in1=r1[64:128, t, :])

            # weave the A2 chain with qT(6)/qT(7) pieces: every cross-engine
            # hop is covered by ~1.7us of independent PE work
            qt_ts(6, [0])
            side_a(xe16, xet_sb)
            qt_ts(6, [1])
            side_a(xf16, xft_sb, on_act=True)
            side_b_mm(wk_t, xet_sb, klr16)
            qt_ts(6, [2])
            side_b_fin(r1k_t, kbd, klr16)
            qt_ts(6, [3])
            side_b_mm(wv_t, xft_sb, vlr16, on_act=True)
            qt_ts(7, [0, 1])
            side_b_fin(r1v_t, vbd, vlr16)
            qt_ts(7, [2, 3])

        a2_cm.__exit__(None, None, None)
        warm_cm.__exit__(None, None, None)

        # A2 step 3: vw (own psum pool; pass B's back_half needs it ~17us out)
        with tc.tile_pool(name="vwps", bufs=2, space="PSUM") as vw_pool:
            for t in range(4):
                for f in (0, 512):
                    vw_ps = vw_pool.tile([128, 512], FP32)
                    nc.tensor.matmul(vw_ps[:], vbd[:, t, :],
                                     wob_t[:, t, f:f + 512],
                                     start=True, stop=True)
                    if f == 0:
                        nc.vector.tensor_copy(vw_sb[:, t, f:f + 512], vw_ps[:])
                    else:
                        nc.scalar.copy(vw_sb[:, t, f:f + 512], vw_ps[:])

        # ---------------- Pass B: dotsT -> softmax -> out --------------------
        # three-deep software pipeline: dots(s) | sums(s-1) | out(s-2), so the
        # PE never waits on the ACT exp or the DVE normalize
        exp_pool = ctx.enter_context(tc.tile_pool(name="expp", bufs=9))
        attn_pool = ctx.enter_context(tc.tile_pool(name="attn", bufs=3))
        osb_pool = ctx.enter_context(tc.tile_pool(name="osb", bufs=3))
        dots_pool = ctx.enter_context(tc.tile_pool(name="dots", bufs=4, space="PSUM"))
        sums_pool = ctx.enter_context(tc.tile_pool(name="sums", bufs=1, space="PSUM"))
        out_ps_pool = ctx.enter_context(tc.tile_pool(name="outps", bufs=2, space="PSUM"))

        exp_tiles = [None] * NSUPER
        attn_tiles = [None] * NSUPER

        def front_dots(s):
            exps = []
            for t in range(4):
                dots_ps = dots_pool.tile([128, 512], FP32)
                nc.tensor.matmul(dots_ps[:], kbd[:, t, :],
                                 qT_sb[:, t, s * 512:(s + 1) * 512],
                                 start=True, stop=True)
                expT = exp_pool.tile([128, 512], BF16)
                nc.scalar.activation(
                    out=expT[:], in_=dots_ps[:],
                    func=mybir.ActivationFunctionType.Exp,
                    scale=0.125, bias=nbias[:],
                )
                exps.append(expT)
            exp_tiles[s] = exps

        def front_sums(s):
            attnT = attn_pool.tile([128, 4, 512], FP16)
            attn_tiles[s] = attnT
            for t in range(4):
                expT = exp_tiles[s][t]
                sums_ps = sums_pool.tile([128, 512], FP32)
                nc.tensor.matmul(sums_ps[:], bones_t[:], expT[:],
                                 start=True, stop=True)
                rec = exp_pool.tile([128, 512], FP32, tag="rec")
                nc.vector.reciprocal_approx_fast(out=rec[:], in_=sums_ps[:])
                nc.vector.tensor_mul(out=attnT[:, t, :], in0=expT[:], in1=rec[:])

        def back_half(s):
            attnT = attn_tiles[s]
            for q in range(4):
                out_sb = osb_pool.tile([128, DIM], FP16)
                for f in (0, 512):
                    out_ps = out_ps_pool.tile([128, 512], FP32)
                    for t in range(4):
                        nc.tensor.matmul(
                            out_ps[:],
                            attnT[:, t, q * 128:(q + 1) * 128],
                            vw_sb[:, t, f:f + 512],
                            start=(t == 0), stop=(t == 3),
                        )
                    # DVE is the trailing engine in pass B (recip+mult), so
                    # out evacuation lives on ACT; the final superblock keeps
                    # the DVE/ACT split to shorten the last copy->DMA chain
                    if s == NSUPER - 1 and f == 0:
                        nc.vector.tensor_copy(out_sb[:, 0:512], out_ps[:])
                    else:
                        nc.scalar.copy(out_sb[:, f:f + 512], out_ps[:])
                i = s * 4 + q
                # sync queue is idle once inputs are loaded; its hardware DGE
                # is much faster than gpsimd's software path
                nc.sync.dma_start(out_p[i * 128:(i + 1) * 128, :], out_sb[:])

        front_dots(0)
        front_dots(1)
        front_sums(0)
        for s in range(2, NSUPER):
            front_dots(s)
            front_sums(s - 1)
            back_half(s - 2)
        front_sums(NSUPER - 1)
        back_half(NSUPER - 2)
        back_half(NSUPER - 1)

    nc.finalize()
    _PROG_CACHE["nc"] = nc
    return nc


def shard_inputs(x, E, F, W_qkv, b_qkv, W_out, b_out):
    x = np.asarray(x, dtype=np.float32)
    E = np.asarray(E, dtype=np.float32)
    F = np.asarray(F, dtype=np.float32)
    W_qkv = np.asarray(W_qkv, dtype=np.float32)
    b_qkv = np.asarray(b_qkv, dtype=np.float32)
    W_out = np.asarray(W_out, dtype=np.float32)

    sE = E.sum(0).astype(np.float32)
    sF = F.sum(0).astype(np.float32)
    EF = np.concatenate([E, F], axis=1).astype(np.float16)
    EFp = np.ascontiguousarray(
        EF.reshape(NCHUNK, 128, 2 * K).transpose(1, 0, 2).reshape(128, -1))

    ident = np.eye(128, dtype=np.float16)
    import ml_dtypes
    bones = np.zeros((128, 128), np.float32)
    bones[:64, :64] = 1.0
    bones[64:, 64:] = 1.0
    bones = bones.astype(ml_dtypes.bfloat16)

    in_maps = []
    xb_cache = {}
    for c in range(NCORES):
        b, hg = c // 2, c % 2
        hs = NH * hg
        if b not in xb_cache:
            xb16 = np.ascontiguousarray(x[:, b, :]).astype(np.float16)
            xT16 = np.ascontiguousarray(xb16.T)
            xb_cache[b] = (xb16, xT16)
        xb16, xT16 = xb_cache[b]

        qcols = slice(hs * DH, (hs + NH) * DH)
        kcols = slice(DIM + hs * DH, DIM + (hs + NH) * DH)
        vcols = slice(2 * DIM + hs * DH, 2 * DIM + (hs + NH) * DH)

        bqp32 = np.ascontiguousarray(
            b_qkv[qcols].reshape(4, 128).T).astype(np.float32)
        bk = b_qkv[kcols]
        bv = b_qkv[vcols]
        r1kT = np.ascontiguousarray(
            (bk.reshape(4, 128)[:, :, None] * sE[None, None, :])
            .transpose(1, 0, 2).reshape(128, 4 * K))
        r1vT = np.ascontiguousarray(
            (bv.reshape(4, 128)[:, :, None] * sF[None, None, :])
            .transpose(1, 0, 2).reshape(128, 4 * K))

        in_maps.append({
            "ident": ident,
            "bones": bones,
            "EFp": EFp,
            "x_nat": xb16,
            "xT": xT16,
            "Wq": W_qkv[:, qcols].astype(np.float16),
            "Wk": W_qkv[:, kcols].astype(np.float16),
            "Wv": W_qkv[:, vcols].astype(np.float16),
            "WoB": W_out[hs * DH:(hs + NH) * DH, :].astype(np.float16),
            "bqp": bqp32,
            "r1k": r1kT.astype(np.float32),
            "r1v": r1vT.astype(np.float32),
        })
    return in_maps


def kernel_impl(inputs, trace=False, **run_kwargs):
    nc = build_program()
    in_maps = shard_inputs(
        inputs["x"], inputs["E"], inputs["F"], inputs["W_qkv"],
        inputs["b_qkv"], inputs["W_out"], inputs["b_out"],
    )
    res = run_bass_kernel_spmd(nc, in_maps, list(range(NCORES)),
                               trace=trace, **run_kwargs)
    b_out = np.asarray(inputs["b_out"], dtype=np.float32)
    out = np.empty((N, B, DIM), np.float32)
    for b in range(B):
        out[:, b, :] = (res.results[2 * b]["out_p"].astype(np.float32)
                        + res.results[2 * b + 1]["out_p"].astype(np.float32)
                        + b_out)
    return out, res


def kernel(**inputs):
    out, _ = kernel_impl(inputs)
    return out
